# revision 1
# baseline (speedup 1.0000x reference)
# GAT (2-layer, 8-head) Trainium2 Bass kernel.
# Data-parallel over batch across 8 NeuronCores (2 batches/core).
#
# Per layer, per batch:
#   P = x @ pw + pb                  (per-head projection, PE)
#   score[i,j] = leaky(a1.P_i + a2.P_j + ab), masked (smask | adj==0) -> NEG
#   attn = softmax(score, -1); out = attn @ P; r = x + concat(out); LN(r)
#
# Device strategy: scores are built directly TRANSPOSED ([j partitions, i free])
# so the attn @ P matmul needs no attention transpose; softmax denominator
# comes free from a ones-column in the P operand; division is applied to the
# (tiny) output instead of the (huge) attention matrix. Masks are combined
# into an additive -30000 fp16 matrix once per batch and reused 2 layers x 8
# heads. exp(leaky(z)) never materializes row maxes (scores are O(1) bounded;
# masked lanes underflow to exactly 0).
import os
import numpy as np
from contextlib import ExitStack

NEGV = -30000.0
LN_EPS = 1e-5

_CACHE = {}
LAST_EXEC_NS = None
ACT_HEADS_DEFAULT = 5
MASK_ON_POOL = False


def _bcast_ap(ap, p=128):
    """Replicate a [free...] AP across p partitions (stride-0 partition dim)."""
    import concourse.bass as bass
    return bass.AP(tensor=ap.tensor, offset=ap.offset, ap=[[0, p]] + list(ap.ap))


def _build(B2, S, M, H, L, semantic, apply_g, reps=1):
    import concourse.bass as bass
    import concourse.bacc as bacc
    import concourse.tile as tile
    from concourse import mybir
    from concourse._compat import axon_active

    f16 = mybir.dt.float16
    f32 = mybir.dt.float32
    Alu = mybir.AluOpType
    Act = mybir.ActivationFunctionType

    DK = M // H
    ST = S // 128          # row tiles (also column tiles)
    KT = M // 128          # contraction tiles for the projection
    HC = H * 35            # packed cols/head: 32 P + 1 one + 1 w2(s2) + 1 w1(s1)
    CH = min(4, ST)        # j-tiles per dense chunk (STT2/exp batch)
    ACT_HEADS = ACT_HEADS_DEFAULT  # heads routed through the ACT (Prelu-bias) path
    NCH = ST // CH

    nc = bacc.Bacc(
        "TRN2", target_bir_lowering=False, debug=not axon_active(), num_devices=8)
    adj_d = nc.declare_dram_parameter("adj", [B2, S, S], mybir.dt.int32, isOutput=False)
    sm_d = nc.declare_dram_parameter("smask", [B2, S, S], mybir.dt.uint8, isOutput=False)
    x0_d = nc.declare_dram_parameter("x0", [B2, S, M], f32, isOutput=False)
    pw_d = nc.declare_dram_parameter("pwcat", [L, KT, 128, HC], f16, isOutput=False)
    bc_d = nc.declare_dram_parameter("biascat", [L, HC], f32, isOutput=False)
    id_d = nc.declare_dram_parameter("ident", [128, 128], f16, isOutput=False)
    if apply_g:
        g_d = nc.declare_dram_parameter("lng", [L, M], f32, isOutput=False)
        b_d = nc.declare_dram_parameter("lnb", [L, M], f32, isOutput=False)
    out_d = nc.declare_dram_parameter("out", [B2, S, M], f32, isOutput=True)

    n_masks = B2 * (2 if semantic else 1)

    with tile.TileContext(nc) as tc, ExitStack() as ctx:
        singles = ctx.enter_context(tc.tile_pool(name="singles", bufs=1))
        persist = ctx.enter_context(tc.tile_pool(name="persist", bufs=1))
        io = ctx.enter_context(tc.tile_pool(name="io", bufs=2))
        maskw = ctx.enter_context(tc.tile_pool(name="maskw", bufs=2))
        dense = ctx.enter_context(tc.tile_pool(name="dense", bufs=2))
        xpool = ctx.enter_context(tc.tile_pool(name="xpool", bufs=4))
        lay = ctx.enter_context(tc.tile_pool(name="lay", bufs=2))
        small = ctx.enter_context(tc.tile_pool(name="small", bufs=4))
        ptrp = ctx.enter_context(tc.tile_pool(name="ptrp", bufs=2, space="PSUM"))
        pprojp = ctx.enter_context(tc.tile_pool(name="pprojp", bufs=2, space="PSUM"))
        dramp = ctx.enter_context(tc.tile_pool(name="dramp", bufs=2, space="DRAM"))
        pavp = ctx.enter_context(tc.tile_pool(name="pavp", bufs=4, space="PSUM"))

        ident = singles.tile([128, 128], f16)
        nc.sync.dma_start(out=ident[:], in_=id_d[:])
        eps_t = singles.tile([128, 1], f32)
        nc.vector.memset(eps_t[:], LN_EPS)

        rep_cm = tc.For_i(
            0, reps, 1, name="rep",
            hint_engines=(mybir.EngineType.PE, mybir.EngineType.DVE,
                          mybir.EngineType.Activation, mybir.EngineType.SP,
                          mybir.EngineType.Pool)) if reps > 1 else None
        if rep_cm is not None:
            ctx.enter_context(rep_cm)

        # ---------------- Stage A: combined transposed masks ----------------
        # MT[b][:, jt, i] = 0 if (adj[b, i, j]!=0 and not smask[b, i, j]) else NEGV
        # (j = jt*128 + partition). Optionally a smask-only variant for
        # semantic layers > 0.
        mts = []
        for b in range(B2):
            variants = [(True, persist.tile([128, ST, S], f16, tag=f"mt{b}", name=f"mt{b}"))]
            if semantic:
                variants.append((False, persist.tile([128, ST, S], f16, tag=f"mtsm{b}", name=f"mtsm{b}")))
            mts.append(variants)
            for use_adj, mt in variants:
                for s in range(ST):
                    sm_t = io.tile([128, S], mybir.dt.uint8, tag="smt")
                    nc.sync.dma_start(out=sm_t[:], in_=sm_d[b, s * 128:(s + 1) * 128, :])
                    ms = maskw.tile([128, S], f16, tag="ms")
                    if use_adj:
                        adj_t = io.tile([128, S], mybir.dt.int32, tag="adjt")
                        nc.sync.dma_start(out=adj_t[:], in_=adj_d[b, s * 128:(s + 1) * 128, :])
                        tneg = maskw.tile([128, S], f16, tag="tneg")
                        if b == 0 or not MASK_ON_POOL:
                            # fast path: get batch 0's mask ready ASAP on DVE
                            nc.vector.scalar_tensor_tensor(
                                out=tneg[:], in0=sm_t[:], scalar=1.0, in1=adj_t[:],
                                op0=Alu.subtract, op1=Alu.mult)
                        else:
                            t1 = maskw.tile([128, S], f16, tag="t1")
                            # t1 = sm*adj ; tneg = t1 - adj == -keep  (on Pool)
                            nc.gpsimd.tensor_tensor(
                                out=t1[:], in0=sm_t[:], in1=adj_t[:], op=Alu.mult)
                            nc.gpsimd.tensor_tensor(
                                out=tneg[:], in0=t1[:], in1=adj_t[:], op=Alu.subtract)
                        # ms = (tneg + 1) * NEGV   (0 where kept, NEGV where masked)
                        nc.vector.tensor_scalar(
                            out=ms[:], in0=tneg[:], scalar1=1.0, scalar2=NEGV,
                            op0=Alu.add, op1=Alu.mult)
                    else:
                        # ms = smask * NEGV
                        nc.vector.tensor_scalar(
                            out=ms[:], in0=sm_t[:], scalar1=NEGV, scalar2=None,
                            op0=Alu.mult)
                    for jt in range(ST):
                        ptr = ptrp.tile([128, 128], f16, tag="ptr")
                        nc.tensor.transpose(ptr[:], ms[:, jt * 128:(jt + 1) * 128], ident[:])
                        eng = nc.vector if (jt % 2 == 0) else nc.scalar
                        if eng is nc.vector:
                            nc.vector.tensor_copy(out=mt[:, jt, s * 128:(s + 1) * 128], in_=ptr[:])
                        else:
                            nc.scalar.copy(out=mt[:, jt, s * 128:(s + 1) * 128], in_=ptr[:])

        # ---------------- x0 load & cast ----------------
        xf16 = {}
        for b in range(B2):
            xf16[(b, 0)] = xpool.tile([128, ST, M], f16, tag="xf16", name=f"xf16_{b}_0")
            for s in range(ST):
                xs = io.tile([128, M], f32, tag="x0s")
                nc.sync.dma_start(out=xs[:], in_=x0_d[b, s * 128:(s + 1) * 128, :])
                nc.vector.tensor_copy(out=xf16[(b, 0)][:, s, :], in_=xs[:])

        # ---------------- Layers ----------------
        for l in range(L):
            pw_sb = [lay.tile([128, HC], f16, tag="pwsb", name=f"pwsb{_}") for _ in range(KT)]
            for kt in range(KT):
                nc.sync.dma_start(out=pw_sb[kt][:], in_=pw_d[l, kt])
            biasb = lay.tile([128, HC], f32, tag="biasb")
            nc.sync.dma_start(out=biasb[:], in_=_bcast_ap(bc_d[l]))
            if apply_g:
                gb = lay.tile([128, M], f32, tag="gb")
                nc.sync.dma_start(out=gb[:], in_=_bcast_ap(g_d[l]))
                bb = lay.tile([128, M], f32, tag="bb")
                nc.sync.dma_start(out=bb[:], in_=_bcast_ap(b_d[l]))

            for b in range(B2):
                x16 = xf16[(b, l)]
                mt = mts[b][1][1] if (semantic and l > 0) else mts[b][0][1]

                # xT (f16, [m, s] layout) via DMA xbar transposes
                xT = lay.tile([128, KT, S], f16, tag="xT")
                for kt in range(KT):
                    for s in range(ST):
                        nc.sync.dma_start_transpose(
                            out=xT[:, kt, s * 128:(s + 1) * 128],
                            in_=x16[:, s, kt * 128:(kt + 1) * 128])

                # Projection: P_sb[:, s, h, 0:32] = P, [...,32] = 1.0, [...,33] = s2
                P_sb = lay.tile([128, ST, H, 35], f16, tag="Psb")
                for s in range(ST):
                    pproj = pprojp.tile([128, HC], f32, tag="pproj")
                    for kt in range(KT):
                        nc.tensor.matmul(
                            pproj[:], xT[:, kt, s * 128:(s + 1) * 128], pw_sb[kt][:],
                            start=(kt == 0), stop=(kt == KT - 1))
                    nc.vector.scalar_tensor_tensor(
                        out=P_sb[:, s, :, :], in0=pproj[:], scalar=0.0, in1=biasb[:],
                        op0=Alu.add, op1=Alu.add)

                # s1 values live in P_sb[:, st, h, 34]; bounce them to DRAM in
                # row-major [h, s] layout so per-head partition broadcasts work.
                s1dram = dramp.tile([H, S], f16, tag="s1dram")
                for st in range(ST):
                    nc.sync.dma_start(
                        out=bass.AP(tensor=s1dram.tensor, offset=s1dram.offset + st * 128,
                                    ap=[[1, 128], [S, H]]),
                        in_=P_sb[:, st, :, 34])

                conc = lay.tile([128, ST, M], f16, tag="conc")
                for h in range(H):
                    act_path = (h % 2 == 0) if ACT_HEADS == 4 else (h < ACT_HEADS)
                    s1b = dense.tile([128, S], f16, tag="s1b", bufs=4)
                    nc.sync.dma_start(out=s1b[:], in_=_bcast_ap(s1dram[h, :]))
                    pav = pavp.tile([128, ST, 36], f32, tag="pav")
                    vs = []
                    for c in range(NCH):
                        z = dense.tile([128, CH, S], f16, tag="z", bufs=3)
                        v = dense.tile([128, CH, S], f16, tag="v", bufs=2 * NCH,
                                       name=f"v{c}")
                        if act_path:
                            for j in range(CH):
                                jt = c * CH + j
                                nc.vector.tensor_tensor(
                                    out=z[:, j, :], in0=mt[:, jt, :],
                                    in1=s1b[:], op=Alu.add)
                            for j in range(CH):
                                jt = c * CH + j
                                nc.scalar.activation(
                                    out=z[:, j, :], in_=z[:, j, :], func=Act.Prelu,
                                    bias=P_sb[:, jt, h, 33:34], alpha=0.2)
                            nc.scalar.activation(out=v[:], in_=z[:], func=Act.Exp)
                        else:
                            for j in range(CH):
                                jt = c * CH + j
                                nc.vector.scalar_tensor_tensor(
                                    out=z[:, j, :], in0=mt[:, jt, :],
                                    scalar=P_sb[:, jt, h, 33:34], in1=s1b[:],
                                    op0=Alu.add, op1=Alu.add)
                            nc.vector.scalar_tensor_tensor(
                                out=v[:], in0=z[:], scalar=0.2, in1=z[:],
                                op0=Alu.mult, op1=Alu.max)
                            nc.scalar.activation(out=v[:], in_=v[:], func=Act.Exp)
                        vs.append(v)
                    for ib in range(ST):
                        for c in range(NCH):
                            for j in range(CH):
                                jt = c * CH + j
                                nc.tensor.matmul(
                                    pav[:, ib, 0:33],
                                    vs[c][:, j, ib * 128:(ib + 1) * 128],
                                    P_sb[:, jt, h, 0:33],
                                    start=(jt == 0), stop=(jt == ST - 1))
                    rec = small.tile([128, ST], f32, tag="rec")
                    nc.vector.reciprocal(out=rec[:], in_=pav[:, :, 32])
                    nc.vector.tensor_tensor(
                        out=conc[:, :, h * DK:(h + 1) * DK],
                        in0=pav[:, :, 0:DK],
                        in1=rec[:].rearrange("p (s one) -> p s one", one=1).broadcast_to([128, ST, DK]),
                        op=Alu.mult)

                # Residual + LayerNorm
                rr = lay.tile([128, ST, M], f16, tag="rr")
                sums = small.tile([128, ST], f32, tag="sums")
                sq = small.tile([128, ST], f32, tag="sq")
                for s in range(ST):
                    nc.vector.scalar_tensor_tensor(
                        out=rr[:, s, :], in0=conc[:, s, :], scalar=0.0, in1=x16[:, s, :],
                        op0=Alu.add, op1=Alu.add, accum_out=sums[:, s:s + 1])
                    scr = small.tile([128, M], f32, tag="scr")
                    nc.scalar.activation(out=scr[:], in_=rr[:, s, :], func=Act.Square,
                                         accum_out=sq[:, s:s + 1])
                mu = small.tile([128, ST], f32, tag="mu")
                nc.vector.tensor_scalar(out=mu[:], in0=sums[:], scalar1=1.0 / M,
                                        scalar2=None, op0=Alu.mult)
                mu2 = small.tile([128, ST], f32, tag="mu2")
                nc.vector.tensor_tensor(out=mu2[:], in0=mu[:], in1=mu[:], op=Alu.mult)
                var = small.tile([128, ST], f32, tag="var")
                nc.vector.scalar_tensor_tensor(
                    out=var[:], in0=sq[:], scalar=1.0 / M, in1=mu2[:],
                    op0=Alu.mult, op1=Alu.subtract)
                # rstd = 1/sqrt(var+eps) via Babylonian iterations + reciprocal
                # (avoids Sqrt/Ln ACT table switches away from the exp set)
                ve = small.tile([128, ST], f32, tag="ve")
                nc.vector.tensor_scalar(out=ve[:], in0=var[:], scalar1=LN_EPS,
                                        scalar2=None, op0=Alu.add)
                std = small.tile([128, ST], f32, tag="std")
                nc.vector.tensor_scalar(out=std[:], in0=ve[:], scalar1=0.4,
                                        scalar2=0.7, op0=Alu.mult, op1=Alu.add)
                for _it in range(3):
                    rs = small.tile([128, ST], f32, tag="rs", name=f"rs{_it}")
                    nc.vector.reciprocal(out=rs[:], in_=std[:])
                    tdiv = small.tile([128, ST], f32, tag="tdiv", name=f"tdiv{_it}")
                    nc.vector.tensor_tensor(out=tdiv[:], in0=ve[:], in1=rs[:],
                                            op=Alu.mult)
                    usum = small.tile([128, ST], f32, tag="usum", name=f"usum{_it}")
                    nc.vector.tensor_tensor(out=usum[:], in0=std[:], in1=tdiv[:],
                                            op=Alu.add)
                    std2 = small.tile([128, ST], f32, tag="std", name=f"std{_it}")
                    nc.vector.tensor_scalar(out=std2[:], in0=usum[:], scalar1=0.5,
                                            scalar2=None, op0=Alu.mult)
                    std = std2
                rstd = small.tile([128, ST], f32, tag="rstd")
                nc.vector.reciprocal(out=rstd[:], in_=std[:])

                last = (l == L - 1)
                if last:
                    y32 = lay.tile([128, ST, M], f32, tag="y32")
                else:
                    xf16[(b, l + 1)] = xpool.tile([128, ST, M], f16, tag="xf16", name=f"xf16_{b}_{l+1}")
                for s in range(ST):
                    if apply_g:
                        tmp = small.tile([128, M], f32, tag="ytmp")
                        nc.vector.tensor_scalar(
                            out=tmp[:], in0=rr[:, s, :], scalar1=mu[:, s:s + 1],
                            scalar2=rstd[:, s:s + 1], op0=Alu.subtract, op1=Alu.mult)
                        tmp2 = small.tile([128, M], f32, tag="ytmp2")
                        nc.vector.tensor_tensor(out=tmp2[:], in0=tmp[:], in1=gb[:], op=Alu.mult)
                        ydst = y32[:, s, :] if last else xf16[(b, l + 1)][:, s, :]
                        nc.vector.tensor_tensor(out=ydst, in0=tmp2[:], in1=bb[:], op=Alu.add)
                    else:
                        ydst = y32[:, s, :] if last else xf16[(b, l + 1)][:, s, :]
                        nc.vector.tensor_scalar(
                            out=ydst, in0=rr[:, s, :], scalar1=mu[:, s:s + 1],
                            scalar2=rstd[:, s:s + 1], op0=Alu.subtract, op1=Alu.mult)
                if last:
                    nc.sync.dma_start(
                        out=out_d[b].rearrange("(s p) m -> p s m", p=128), in_=y32[:])
    nc.compile()
    return nc


def _get_nc(key):
    if key not in _CACHE:
        _CACHE[key] = _build(*key)
    return _CACHE[key]


def _pack_weights(proj_w, proj_b, attn_w, attn_b):
    L, H, M, DK = proj_w.shape
    KT = M // 128
    HC = H * 35
    pwcat = np.zeros((L, M, H, 35), np.float32)
    biascat = np.zeros((L, H, 35), np.float32)
    for l in range(L):
        a1, a2 = attn_w[l, :DK], attn_w[l, DK:]
        for h in range(H):
            pwcat[l, :, h, :32] = proj_w[l, h]
            pwcat[l, :, h, 33] = proj_w[l, h] @ a2
            pwcat[l, :, h, 34] = proj_w[l, h] @ a1
            biascat[l, h, :32] = proj_b[l, h]
            biascat[l, h, 32] = 1.0
            biascat[l, h, 33] = proj_b[l, h] @ a2
            biascat[l, h, 34] = proj_b[l, h] @ a1 + attn_b[l]
    return (pwcat.reshape(L, KT, 128, HC).astype(np.float16),
            biascat.reshape(L, HC))


def _prepare(adj, inputs, score_mask, type, proj_w, proj_b, attn_w, attn_b, ln_g, ln_b):
    adj = np.asarray(adj)
    inputs = np.asarray(inputs, dtype=np.float32)
    score_mask = np.asarray(score_mask)
    proj_w = np.asarray(proj_w, dtype=np.float32)
    proj_b = np.asarray(proj_b, dtype=np.float32)
    attn_w = np.asarray(attn_w, dtype=np.float32)
    attn_b = np.asarray(attn_b, dtype=np.float32)
    ln_g = np.asarray(ln_g, dtype=np.float32)
    ln_b = np.asarray(ln_b, dtype=np.float32)

    B, S, M = inputs.shape
    L, H = proj_w.shape[0], proj_w.shape[1]
    NCORES = 8
    B2 = B // NCORES
    semantic = bool(np.asarray(type) == 1)
    apply_g = not (np.allclose(ln_g, 1.0) and np.allclose(ln_b, 0.0))

    pwcat, biascat = _pack_weights(proj_w, proj_b, attn_w, attn_b)
    ident = np.eye(128, dtype=np.float16)
    sm_u8 = np.ascontiguousarray(score_mask[:, 0]).astype(np.uint8)
    adj_i32 = np.ascontiguousarray(adj.astype(np.int32))

    in_maps = []
    for c in range(NCORES):
        m = {
            "adj": adj_i32[c * B2:(c + 1) * B2],
            "smask": sm_u8[c * B2:(c + 1) * B2],
            "x0": np.ascontiguousarray(inputs[c * B2:(c + 1) * B2]),
            "pwcat": pwcat, "biascat": biascat, "ident": ident,
        }
        if apply_g:
            m["lng"] = ln_g
            m["lnb"] = ln_b
        in_maps.append(m)

    return (B2, S, M, H, L, semantic, apply_g), in_maps


def kernel(**inputs):
    from concourse.bass_utils import run_bass_kernel_spmd
    key, in_maps = _prepare(**inputs)
    nc = _get_nc(key)
    res = run_bass_kernel_spmd(nc, in_maps, core_ids=list(range(len(in_maps))),
                               trace=bool(int(os.environ.get("GAT_TRACE", "0"))))
    global LAST_EXEC_NS
    LAST_EXEC_NS = res.exec_time_ns
    out = np.concatenate([r["out"] for r in res.results], axis=0)
    return out.astype(np.float32)


def measure_hw_s(reps=64, n_runs=3, **inputs):
    """Estimate per-iteration device time by timing a reps-looped variant
    against the reps=1 variant (amortizes axon dispatch + transfer)."""
    import time
    from concourse.bass_utils import run_bass_kernel_spmd
    key, in_maps = _prepare(**inputs)
    cores = list(range(len(in_maps)))
    nc1 = _get_nc(key)
    ncR = _build(*key, reps=reps)

    def timed(nc):
        best = None
        for _ in range(n_runs):
            t0 = time.time()
            run_bass_kernel_spmd(nc, in_maps, core_ids=cores)
            dt = time.time() - t0
            best = dt if best is None else min(best, dt)
        return best

    t1 = timed(nc1)
    tR = timed(ncR)
    per_iter = (tR - t1) / (reps - 1)
    return per_iter, t1, tR



# revision 3
# speedup vs baseline: 17.6928x; 17.6928x over previous
# GAT (2-layer, 8-head) Trainium2 Bass kernel, v2.
# Data-parallel over batch across 8 NeuronCores (2 batches/core).
#
# Score factorization: with z_ij = s1_i + s2_j + ab,
#   exp(leaky_0.2(z)) = max(exp(z), exp(0.2 z))
#                     = exp(s1_i) * max(w_j, c_i * w2_j)
# where w_j = exp(s2_j+ab), w2_j = exp(0.2(s2_j+ab)), c_i = exp(-0.8 s1_i).
# The exp(s1_i) factor cancels in the softmax, so the S x S work reduces to
#   vtil[j,i] = keepneg[j,i] * max(w_j, c_i*w2_j)
# (keepneg in {-1, 0}; the sign cancels in the softmax division too). Only
# per-node exponentials are needed -- no S x S exp/prelu on ACT.
# The two S x S passes per head are split across ACT/DVE/Pool per HEAD_CFG.
import os
import numpy as np
from contextlib import ExitStack

LN_EPS = 1e-5

_CACHE = {}
LAST_EXEC_NS = None

# per-head (pass1, pass2) engine config:
#  pass1: 'a' = ACT copy-scale (A = c*w2_j), 'd' = DVE ts2 (G = max(c*w2_j, w_j))
#  pass2: for 'a' pass1: stt on DVE ('d'); for 'd' pass1 (G form): tt on DVE ('d')
#         or Pool ('p'). (TensorScalarPtr is NOT legal on Pool; tensor_tensor is.)
HEAD_CFG = [('a', 'd'), ('a', 'd'), ('a', 'd'), ('a', 'd'),
            ('d', 'p'), ('d', 'p'), ('d', 'd'), ('d', 'd')]


def _bcast_ap(ap, p=128):
    """Replicate a [free...] AP across p partitions (stride-0 partition dim)."""
    import concourse.bass as bass
    return bass.AP(tensor=ap.tensor, offset=ap.offset, ap=[[0, p]] + list(ap.ap))


def _build(B2, S, M, H, L, semantic, apply_g, reps=1):
    import concourse.bass as bass
    import concourse.bacc as bacc
    import concourse.tile as tile
    from concourse import mybir
    from concourse._compat import axon_active

    f16 = mybir.dt.float16
    f32 = mybir.dt.float32
    Alu = mybir.AluOpType
    Act = mybir.ActivationFunctionType

    DK = M // H
    ST = S // 128          # row tiles (also column tiles)
    KT = M // 128          # contraction tiles for the projection
    HCW = 36               # cols/head: 32 P, ones, s2raw, s1raw, c(f16)
    HC = H * HCW
    CH = min(4, ST)        # j-tiles per dense chunk
    NCH = ST // CH

    nc = bacc.Bacc(
        "TRN2", target_bir_lowering=False, debug=not axon_active(), num_devices=8)
    adj_d = nc.declare_dram_parameter("adj", [B2, S, S], mybir.dt.int32, isOutput=False)
    sm_d = nc.declare_dram_parameter("smask", [B2, S, S], mybir.dt.uint8, isOutput=False)
    x0_d = nc.declare_dram_parameter("x0", [B2, S, M], f32, isOutput=False)
    pw_d = nc.declare_dram_parameter("pwcat", [L, KT, 128, HC], f16, isOutput=False)
    bc_d = nc.declare_dram_parameter("biascat", [L, HC], f32, isOutput=False)
    id_d = nc.declare_dram_parameter("ident", [128, 128], f16, isOutput=False)
    if apply_g:
        g_d = nc.declare_dram_parameter("lng", [L, M], f32, isOutput=False)
        b_d = nc.declare_dram_parameter("lnb", [L, M], f32, isOutput=False)
    out_d = nc.declare_dram_parameter("out", [B2, S, M], f32, isOutput=True)

    with tile.TileContext(nc) as tc, ExitStack() as ctx:
        singles = ctx.enter_context(tc.tile_pool(name="singles", bufs=1))
        persist = ctx.enter_context(tc.tile_pool(name="persist", bufs=1))
        io = ctx.enter_context(tc.tile_pool(name="io", bufs=2))
        maskw = ctx.enter_context(tc.tile_pool(name="maskw", bufs=2))
        dense = ctx.enter_context(tc.tile_pool(name="dense", bufs=2))
        xpool = ctx.enter_context(tc.tile_pool(name="xpool", bufs=4))
        lay = ctx.enter_context(tc.tile_pool(name="lay", bufs=2))
        small = ctx.enter_context(tc.tile_pool(name="small", bufs=4))
        ptrp = ctx.enter_context(tc.tile_pool(name="ptrp", bufs=2, space="PSUM"))
        pprojp = ctx.enter_context(tc.tile_pool(name="pprojp", bufs=2, space="PSUM"))
        dramp = ctx.enter_context(tc.tile_pool(name="dramp", bufs=2, space="DRAM"))
        pavp = ctx.enter_context(tc.tile_pool(name="pavp", bufs=4, space="PSUM"))

        ident = singles.tile([128, 128], f16)
        nc.sync.dma_start(out=ident[:], in_=id_d[:])

        rep_cm = tc.For_i(
            0, reps, 1, name="rep",
            hint_engines=(mybir.EngineType.PE, mybir.EngineType.DVE,
                          mybir.EngineType.Activation, mybir.EngineType.SP,
                          mybir.EngineType.Pool)) if reps > 1 else None
        if rep_cm is not None:
            ctx.enter_context(rep_cm)

        # ---------------- Stage A: transposed multiplicative masks ----------
        # kp[b][:, jt, i] = -1 if (adj[b, i, j]!=0 and not smask[b, i, j]) else 0
        # (j = jt*128 + partition). Sign cancels in the softmax division.
        kps = []
        for b in range(B2):
            variants = [(True, persist.tile([128, ST, S], f16, tag=f"kp{b}", name=f"kp{b}"))]
            if semantic:
                variants.append((False, persist.tile([128, ST, S], f16, tag=f"kpsm{b}", name=f"kpsm{b}")))
            kps.append(variants)
            for use_adj, kp in variants:
                for s in range(ST):
                    sm_t = io.tile([128, S], mybir.dt.uint8, tag="smt")
                    nc.sync.dma_start(out=sm_t[:], in_=sm_d[b, s * 128:(s + 1) * 128, :])
                    ms = maskw.tile([128, S], f16, tag="ms")
                    if use_adj:
                        adj_t = io.tile([128, S], mybir.dt.int32, tag="adjt")
                        nc.sync.dma_start(out=adj_t[:], in_=adj_d[b, s * 128:(s + 1) * 128, :])
                        # ms = (sm - 1) * adj  in {-1, 0}; -1 marks kept edges
                        nc.vector.scalar_tensor_tensor(
                            out=ms[:], in0=sm_t[:], scalar=1.0, in1=adj_t[:],
                            op0=Alu.subtract, op1=Alu.mult)
                    else:
                        # ms = sm - 1  in {-1, 0}
                        nc.vector.tensor_scalar(
                            out=ms[:], in0=sm_t[:], scalar1=1.0, scalar2=None,
                            op0=Alu.subtract)
                    for jt in range(ST):
                        ptr = ptrp.tile([128, 128], f16, tag="ptr")
                        nc.tensor.transpose(ptr[:], ms[:, jt * 128:(jt + 1) * 128], ident[:])
                        if jt % 2 == 0:
                            nc.vector.tensor_copy(out=kp[:, jt, s * 128:(s + 1) * 128], in_=ptr[:])
                        else:
                            nc.scalar.copy(out=kp[:, jt, s * 128:(s + 1) * 128], in_=ptr[:])

        # ---------------- x0 load & cast ----------------
        xf16 = {}
        for b in range(B2):
            xf16[(b, 0)] = xpool.tile([128, ST, M], f16, tag="xf16", name=f"xf16_{b}_0")
            for s in range(ST):
                xs = io.tile([128, M], f32, tag="x0s")
                nc.sync.dma_start(out=xs[:], in_=x0_d[b, s * 128:(s + 1) * 128, :])
                nc.vector.tensor_copy(out=xf16[(b, 0)][:, s, :], in_=xs[:])

        # ---------------- Layers ----------------
        for l in range(L):
            pw_sb = [lay.tile([128, HC], f16, tag="pwsb", name=f"pwsb{_}") for _ in range(KT)]
            for kt in range(KT):
                nc.sync.dma_start(out=pw_sb[kt][:], in_=pw_d[l, kt])
            biasb = lay.tile([128, HC], f32, tag="biasb")
            nc.sync.dma_start(out=biasb[:], in_=_bcast_ap(bc_d[l]))
            if apply_g:
                gb = lay.tile([128, M], f32, tag="gb")
                nc.sync.dma_start(out=gb[:], in_=_bcast_ap(g_d[l]))
                bb = lay.tile([128, M], f32, tag="bb")
                nc.sync.dma_start(out=bb[:], in_=_bcast_ap(b_d[l]))

            for b in range(B2):
                x16 = xf16[(b, l)]
                kp = kps[b][1][1] if (semantic and l > 0) else kps[b][0][1]

                # xT (f16, [m, s] layout) via DMA xbar transposes
                xT = lay.tile([128, KT, S], f16, tag="xT")
                for kt in range(KT):
                    for s in range(ST):
                        nc.sync.dma_start_transpose(
                            out=xT[:, kt, s * 128:(s + 1) * 128],
                            in_=x16[:, s, kt * 128:(kt + 1) * 128])

                # Projection -> P_sb[:, s, h, :]: [0:32]=P, [32]=1, [33]=s2+ab,
                # [34]=s1; ACT then fills [35]=c (f16) and sc_sb w/w2 (f32).
                P_sb = lay.tile([128, ST, H, HCW], f16, tag="Psb")
                for s in range(ST):
                    pproj = pprojp.tile([128, HC], f32, tag="pproj")
                    for kt in range(KT):
                        nc.tensor.matmul(
                            pproj[:], xT[:, kt, s * 128:(s + 1) * 128], pw_sb[kt][:],
                            start=(kt == 0), stop=(kt == KT - 1))
                    nc.vector.scalar_tensor_tensor(
                        out=P_sb[:, s, :, :], in0=pproj[:], scalar=0.0, in1=biasb[:],
                        op0=Alu.add, op1=Alu.add)
                # per-node exponentials (tiny, [128, ST*H] strided)
                sc_sb = lay.tile([128, ST, H, 2], f32, tag="scsb")
                nc.scalar.activation(out=sc_sb[:, :, :, 0], in_=P_sb[:, :, :, 33],
                                     func=Act.Exp)
                nc.scalar.activation(out=sc_sb[:, :, :, 1], in_=P_sb[:, :, :, 33],
                                     func=Act.Exp, scale=0.2)
                nc.scalar.activation(out=P_sb[:, :, :, 35], in_=P_sb[:, :, :, 34],
                                     func=Act.Exp, scale=-0.8)

                # bounce c to DRAM row-major [H, S] for partition broadcast
                cw = dramp.tile([H, S], f16, tag="cw")
                for st in range(ST):
                    nc.sync.dma_start(
                        out=bass.AP(tensor=cw.tensor, offset=cw.offset + st * 128,
                                    ap=[[1, 128], [S, H]]),
                        in_=P_sb[:, st, :, 35])

                conc = lay.tile([128, ST, M], f16, tag="conc")
                for h in range(H):
                    p1eng, p2eng = HEAD_CFG[h]
                    cb = dense.tile([128, S], f16, tag="cb", bufs=4)
                    nc.sync.dma_start(out=cb[:], in_=_bcast_ap(cw[h, :]))
                    pav = pavp.tile([128, ST, 36], f32, tag="pav")
                    vs = []
                    for c in range(NCH):
                        v = dense.tile([128, CH, S], f16, tag="v", bufs=2 * NCH,
                                       name=f"v{c}")
                        if p1eng == 'a':
                            # A = c_i * w2_j  (ACT copy with per-partition scale)
                            A = dense.tile([128, CH, S], f16, tag="z", bufs=3)
                            for j in range(CH):
                                jt = c * CH + j
                                nc.scalar.activation(
                                    out=A[:, j, :], in_=cb[:], func=Act.Copy,
                                    scale=sc_sb[:, jt, h, 1:2])
                            # v = (A max w_j) * keepneg
                            for j in range(CH):
                                jt = c * CH + j
                                eng = nc.vector if p2eng == 'd' else nc.gpsimd
                                eng.scalar_tensor_tensor(
                                    out=v[:, j, :], in0=A[:, j, :],
                                    scalar=sc_sb[:, jt, h, 0:1],
                                    in1=kp[:, jt, :], op0=Alu.max, op1=Alu.mult)
                        else:
                            # G = (c_i * w2_j) max w_j   (ts with two AP scalars)
                            G = dense.tile([128, CH, S], f16, tag="z", bufs=3)
                            for j in range(CH):
                                jt = c * CH + j
                                nc.vector.tensor_scalar(
                                    out=G[:, j, :], in0=cb[:],
                                    scalar1=sc_sb[:, jt, h, 1:2],
                                    scalar2=sc_sb[:, jt, h, 0:1],
                                    op0=Alu.mult, op1=Alu.max)
                            # v = G * keepneg (one fp16 2x tensor_tensor per chunk)
                            eng2 = nc.vector if p2eng == 'd' else nc.gpsimd
                            eng2.tensor_tensor(
                                out=v[:], in0=G[:],
                                in1=kp[:, c * CH:(c + 1) * CH, :], op=Alu.mult)
                        vs.append(v)
                    for ib in range(ST):
                        for c in range(NCH):
                            for j in range(CH):
                                jt = c * CH + j
                                nc.tensor.matmul(
                                    pav[:, ib, 0:33],
                                    vs[c][:, j, ib * 128:(ib + 1) * 128],
                                    P_sb[:, jt, h, 0:33],
                                    start=(jt == 0), stop=(jt == ST - 1))
                    rec = small.tile([128, ST], f32, tag="rec")
                    nc.vector.reciprocal(out=rec[:], in_=pav[:, :, 32])
                    nc.vector.tensor_tensor(
                        out=conc[:, :, h * DK:(h + 1) * DK],
                        in0=pav[:, :, 0:DK],
                        in1=rec[:].rearrange("p (s one) -> p s one", one=1).broadcast_to([128, ST, DK]),
                        op=Alu.mult)

                # Residual + LayerNorm
                rr = lay.tile([128, ST, M], f16, tag="rr")
                sums = small.tile([128, ST], f32, tag="sums")
                sq = small.tile([128, ST], f32, tag="sq")
                for s in range(ST):
                    nc.vector.scalar_tensor_tensor(
                        out=rr[:, s, :], in0=conc[:, s, :], scalar=0.0, in1=x16[:, s, :],
                        op0=Alu.add, op1=Alu.add, accum_out=sums[:, s:s + 1])
                    scr = small.tile([128, M], f32, tag="scr")
                    nc.scalar.activation(out=scr[:], in_=rr[:, s, :], func=Act.Square,
                                         accum_out=sq[:, s:s + 1])
                mu = small.tile([128, ST], f32, tag="mu")
                nc.vector.tensor_scalar(out=mu[:], in0=sums[:], scalar1=1.0 / M,
                                        scalar2=None, op0=Alu.mult)
                mu2 = small.tile([128, ST], f32, tag="mu2")
                nc.vector.tensor_tensor(out=mu2[:], in0=mu[:], in1=mu[:], op=Alu.mult)
                var = small.tile([128, ST], f32, tag="var")
                nc.vector.scalar_tensor_tensor(
                    out=var[:], in0=sq[:], scalar=1.0 / M, in1=mu2[:],
                    op0=Alu.mult, op1=Alu.subtract)
                # rstd = 1/sqrt(var+eps) via Babylonian iterations + reciprocal
                ve = small.tile([128, ST], f32, tag="ve")
                nc.vector.tensor_scalar(out=ve[:], in0=var[:], scalar1=LN_EPS,
                                        scalar2=None, op0=Alu.add)
                std = small.tile([128, ST], f32, tag="std")
                nc.vector.tensor_scalar(out=std[:], in0=ve[:], scalar1=0.4,
                                        scalar2=0.7, op0=Alu.mult, op1=Alu.add)
                for _it in range(3):
                    rs = small.tile([128, ST], f32, tag="rs", name=f"rs{_it}")
                    nc.vector.reciprocal(out=rs[:], in_=std[:])
                    tdiv = small.tile([128, ST], f32, tag="tdiv", name=f"tdiv{_it}")
                    nc.vector.tensor_tensor(out=tdiv[:], in0=ve[:], in1=rs[:],
                                            op=Alu.mult)
                    usum = small.tile([128, ST], f32, tag="usum", name=f"usum{_it}")
                    nc.vector.tensor_tensor(out=usum[:], in0=std[:], in1=tdiv[:],
                                            op=Alu.add)
                    std2 = small.tile([128, ST], f32, tag="std", name=f"std{_it}")
                    nc.vector.tensor_scalar(out=std2[:], in0=usum[:], scalar1=0.5,
                                            scalar2=None, op0=Alu.mult)
                    std = std2
                rstd = small.tile([128, ST], f32, tag="rstd")
                nc.vector.reciprocal(out=rstd[:], in_=std[:])

                last = (l == L - 1)
                if last:
                    y32 = lay.tile([128, ST, M], f32, tag="y32")
                else:
                    xf16[(b, l + 1)] = xpool.tile([128, ST, M], f16, tag="xf16", name=f"xf16_{b}_{l+1}")
                for s in range(ST):
                    if apply_g:
                        tmp = small.tile([128, M], f32, tag="ytmp")
                        nc.vector.tensor_scalar(
                            out=tmp[:], in0=rr[:, s, :], scalar1=mu[:, s:s + 1],
                            scalar2=rstd[:, s:s + 1], op0=Alu.subtract, op1=Alu.mult)
                        tmp2 = small.tile([128, M], f32, tag="ytmp2")
                        nc.vector.tensor_tensor(out=tmp2[:], in0=tmp[:], in1=gb[:], op=Alu.mult)
                        ydst = y32[:, s, :] if last else xf16[(b, l + 1)][:, s, :]
                        nc.vector.tensor_tensor(out=ydst, in0=tmp2[:], in1=bb[:], op=Alu.add)
                    else:
                        ydst = y32[:, s, :] if last else xf16[(b, l + 1)][:, s, :]
                        nc.vector.tensor_scalar(
                            out=ydst, in0=rr[:, s, :], scalar1=mu[:, s:s + 1],
                            scalar2=rstd[:, s:s + 1], op0=Alu.subtract, op1=Alu.mult)
                if last:
                    nc.sync.dma_start(
                        out=out_d[b].rearrange("(s p) m -> p s m", p=128), in_=y32[:])
    nc.compile()
    return nc


def _get_nc(key):
    if key not in _CACHE:
        _CACHE[key] = _build(*key)
    return _CACHE[key]


def _pack_weights(proj_w, proj_b, attn_w, attn_b):
    L, H, M, DK = proj_w.shape
    KT = M // 128
    HCW = 36
    HC = H * HCW
    pwcat = np.zeros((L, M, H, HCW), np.float32)
    biascat = np.zeros((L, H, HCW), np.float32)
    for l in range(L):
        a1, a2 = attn_w[l, :DK], attn_w[l, DK:]
        for h in range(H):
            pwcat[l, :, h, :32] = proj_w[l, h]
            pwcat[l, :, h, 33] = proj_w[l, h] @ a2
            pwcat[l, :, h, 34] = proj_w[l, h] @ a1
            biascat[l, h, :32] = proj_b[l, h]
            biascat[l, h, 32] = 1.0
            biascat[l, h, 33] = proj_b[l, h] @ a2 + attn_b[l]
            biascat[l, h, 34] = proj_b[l, h] @ a1
    return (pwcat.reshape(L, KT, 128, HC).astype(np.float16),
            biascat.reshape(L, HC))


def _prepare(adj, inputs, score_mask, type, proj_w, proj_b, attn_w, attn_b, ln_g, ln_b):
    adj = np.asarray(adj)
    inputs = np.asarray(inputs, dtype=np.float32)
    score_mask = np.asarray(score_mask)
    proj_w = np.asarray(proj_w, dtype=np.float32)
    proj_b = np.asarray(proj_b, dtype=np.float32)
    attn_w = np.asarray(attn_w, dtype=np.float32)
    attn_b = np.asarray(attn_b, dtype=np.float32)
    ln_g = np.asarray(ln_g, dtype=np.float32)
    ln_b = np.asarray(ln_b, dtype=np.float32)

    B, S, M = inputs.shape
    L, H = proj_w.shape[0], proj_w.shape[1]
    NCORES = 8
    B2 = B // NCORES
    semantic = bool(np.asarray(type) == 1)
    apply_g = not (np.allclose(ln_g, 1.0) and np.allclose(ln_b, 0.0))

    pwcat, biascat = _pack_weights(proj_w, proj_b, attn_w, attn_b)
    ident = np.eye(128, dtype=np.float16)
    sm_u8 = np.ascontiguousarray(score_mask[:, 0]).astype(np.uint8)
    adj_i32 = np.ascontiguousarray(adj.astype(np.int32))

    in_maps = []
    for c in range(NCORES):
        m = {
            "adj": adj_i32[c * B2:(c + 1) * B2],
            "smask": sm_u8[c * B2:(c + 1) * B2],
            "x0": np.ascontiguousarray(inputs[c * B2:(c + 1) * B2]),
            "pwcat": pwcat, "biascat": biascat, "ident": ident,
        }
        if apply_g:
            m["lng"] = ln_g
            m["lnb"] = ln_b
        in_maps.append(m)

    return (B2, S, M, H, L, semantic, apply_g), in_maps


def kernel(**inputs):
    from concourse.bass_utils import run_bass_kernel_spmd
    key, in_maps = _prepare(**inputs)
    nc = _get_nc(key)
    res = run_bass_kernel_spmd(nc, in_maps, core_ids=list(range(len(in_maps))),
                               trace=bool(int(os.environ.get("GAT_TRACE", "0"))))
    global LAST_EXEC_NS
    LAST_EXEC_NS = res.exec_time_ns
    out = np.concatenate([r["out"] for r in res.results], axis=0)
    return out.astype(np.float32)


def measure_hw_s(reps=64, n_runs=3, **inputs):
    import time
    from concourse.bass_utils import run_bass_kernel_spmd
    key, in_maps = _prepare(**inputs)
    cores = list(range(len(in_maps)))
    nc1 = _get_nc(key)
    ncR = _build(*key, reps=reps)

    def timed(nc):
        best = None
        for _ in range(n_runs):
            t0 = time.time()
            run_bass_kernel_spmd(nc, in_maps, core_ids=cores)
            dt = time.time() - t0
            best = dt if best is None else min(best, dt)
        return best

    t1 = timed(nc1)
    tR = timed(ncR)
    per_iter = (tR - t1) / (reps - 1)
    return per_iter, t1, tR
